# revision 6
# baseline (speedup 1.0000x reference)
"""EntityGuidedCrossAttention TRN2 kernel (8 NeuronCores, data-parallel over classes).

Math restructure (exact): labels are contiguous per class, so attention is
block-diagonal.  With folded weights (host-side, weights-only algebra):
    Wqk = Wq^T Wk,  bqk = bq Wk          ->  Qk = ent @ Wqk + bqk
    Wvo = Wv^T Wo^T, bvo = bv Wo^T + bo  ->  OUT = pooled @ Wvo + bvo
    score[c,k] = Qk[c] . sup[c*K+k] / sqrt(D)   (bk is softmax-shift-invariant)
    pooled[c]  = sum_k softmax_w[c,k] * sup[c*K+k]
    res        = sup + OUT[class(row)]

v2 (from trace analysis of the v1 135us kernel):
  - v1 serialized: input DMA (0-45us), BCD+E compute (DMA idle 50-100us),
    then F + ALL output DMA crammed at 100-141us.  PE was also duty-cycle
    throttled (HAM k=4/8) and ACT was ~76us busy on PSUM drains.
  - v2 splits the 32 row-tiles into two halves with per-half E
    (pooled->OUT) and F (residual broadcast+add) so the first half's
    output DMA overlaps the second half's compute.  F(h0) is interleaved
    instruction-by-instruction with B/C/D(h1).
  - Score dots read qkb straight out of PSUM on DVE (the 3-operand
    scalar_tensor_tensor has no DVE 2x/4x modes, so a bf16 staging drain
    buys nothing); ~1/4 of score tiles and F tiles are offloaded via an
    ACT drain to the otherwise-idle GPSIMD engine.
  - Per-class softmax normalizer r comes from ONE matmul per half over a
    densely-packed weight bank (v1 used 32 single-column matmuls).
  - PSUM: one [128,1024]f32 ring (bufs=2) shared by A/qkb/ob/E-out +
    one [64,1024]f32 pooled accumulator + small r/transpose tiles.
"""

import numpy as np

N_CLASSES = 512
K_SHOTS = 64
D = 1024
NK = N_CLASSES * K_SHOTS
N_CORES = 8
C_LOC = N_CLASSES // N_CORES          # 64 classes per core
R_LOC = NK // N_CORES                 # 4096 support rows per core
P = 128
TILES = R_LOC // P                    # 32 row-tiles of 128
DCH = D // P                          # 8 contraction chunks
HTILES = TILES // 2                   # 16 tiles per half
GSZ = 8                               # tiles per softmax group
CPH = 32                              # classes per half
INV_SQRT_D = 1.0 / float(np.sqrt(D))
WPAD = 34                             # w_all per-tile column pitch (32 + 2)
WHALF = WPAD * HTILES                 # w_all columns per half

_NC_CACHE = None


def _build_nc():
    import concourse.bacc as bacc
    import concourse.tile as tile
    import concourse.mybir as mybir
    from concourse.masks import make_identity

    f32 = mybir.dt.float32
    bf16 = mybir.dt.bfloat16
    ADD = mybir.AluOpType.add
    MUL = mybir.AluOpType.mult
    EXP = mybir.ActivationFunctionType.Exp
    CPY = mybir.ActivationFunctionType.Copy

    nc = bacc.Bacc("TRN2", target_bir_lowering=False, debug=False,
                   num_devices=N_CORES)

    sup_d = nc.dram_tensor("sup", [R_LOC, D], bf16, kind="ExternalInput").ap()
    entt_d = nc.dram_tensor("entt", [D, C_LOC], bf16, kind="ExternalInput").ap()
    ind_d = nc.dram_tensor("ind", [C_LOC, R_LOC], bf16, kind="ExternalInput").ap()
    wqk_d = nc.dram_tensor("wqk", [D, D], bf16, kind="ExternalInput").ap()
    wvo_d = nc.dram_tensor("wvo", [D, D], bf16, kind="ExternalInput").ap()
    bqk_d = nc.dram_tensor("bqk", [1, D], bf16, kind="ExternalInput").ap()
    bvo_d = nc.dram_tensor("bvo", [1, D], bf16, kind="ExternalInput").ap()
    res_d = nc.dram_tensor("res", [R_LOC, D], bf16, kind="ExternalOutput").ap()

    with tile.TileContext(nc) as tc:
        with (
            tc.tile_pool(name="const", bufs=1) as const,
            tc.tile_pool(name="sbB", bufs=2) as sbB,
            tc.tile_pool(name="psQ", bufs=2, space="PSUM") as psQ,
            tc.tile_pool(name="psP", bufs=1, space="PSUM") as psP,
            tc.tile_pool(name="psS", bufs=1, space="PSUM") as psS,
            tc.tile_pool(name="psT", bufs=1, space="PSUM") as psT,
        ):
            idf = const.tile([P, P], f32)
            make_identity(nc, idf)
            idb = const.tile([P, P], bf16)
            nc.scalar.copy(out=idb, in_=idf)
            ones_b = const.tile([1, C_LOC], bf16)
            nc.vector.memset(ones_b, 1.0)
            ones_col = const.tile([P, 1], bf16)
            nc.vector.memset(ones_col, 1.0)

            entt_sb = const.tile([P, DCH * C_LOC], bf16)
            ind_sb = const.tile([C_LOC, R_LOC], bf16)
            wqk_sb = const.tile([P, DCH * D], bf16)
            wvo_sb = const.tile([P, DCH * D], bf16)
            bqk_sb = const.tile([1, D], bf16)
            bvo_sb = const.tile([1, D], bf16)
            qk_sb = const.tile([C_LOC, D], bf16)
            out_sb = const.tile([C_LOC, D], bf16)
            pooled_sb = const.tile([C_LOC, D], bf16)
            # chunk ch / half h lhsT block at col 64*ch + 32*h
            pooledt_sb = const.tile([P, DCH * C_LOC], bf16)
            sup_all = const.tile([P, TILES * D], bf16)
            ri_sb = const.tile([C_LOC, 1], f32)
            # padded softmax-weight lhsT bank: within half h, tile j's two
            # columns live at WHALF*h + WPAD*j (+1); its D-matmul lhsT
            # window is [WHALF*h + 32*j, +32) -- only tile j's pair lands
            # inside its own window, everything else there is zero.
            w_all = const.tile([P, 2 * WHALF], bf16)
            nc.vector.memset(w_all, 0.0)
            # densely packed copy (col 32*h + 2*j (+1)) for the single
            # per-half normalizer matmul r = w_r^T @ ones.
            w_r = const.tile([P, C_LOC], bf16)
            nc.vector.memset(w_r, 0.0)

            # ---------------- input DMAs (all bf16, no staging) ------------
            # one hardware queue drains these in issue order across 16 DMA
            # engines at ~330GB/s: consts -> wqk -> ind -> sup h0 -> wvo ->
            # sup h1.  wvo sits between the halves: it is first needed by
            # E(h0) which runs at about that point in the stream.
            nc.sync.dma_start(out=bqk_sb, in_=bqk_d)
            nc.sync.dma_start(out=bvo_sb, in_=bvo_d)
            nc.sync.dma_start(
                out=entt_sb.rearrange("p (ch c) -> p ch c", ch=DCH),
                in_=entt_d.rearrange("(ch p) c -> p ch c", p=P),
            )
            wqk_v = wqk_sb.rearrange("p (ch d) -> p ch d", ch=DCH)
            wqkd_v = wqk_d.rearrange("(ch p) d -> p ch d", p=P)
            for h4 in range(4):
                nc.sync.dma_start(out=wqk_v[:, 2 * h4:2 * h4 + 2, :],
                                  in_=wqkd_v[:, 2 * h4:2 * h4 + 2, :])
            nc.sync.dma_start(out=ind_sb, in_=ind_d)
            sup_v = sup_all.rearrange("p (t d) -> p t d", d=D)
            supd_v = sup_d.rearrange("(t p) d -> p t d", p=P)
            for k in range(8):
                nc.sync.dma_start(out=sup_v[:, 2 * k:2 * k + 2, :],
                                  in_=supd_v[:, 2 * k:2 * k + 2, :])
            wvo_v = wvo_sb.rearrange("p (ch d) -> p ch d", ch=DCH)
            wvod_v = wvo_d.rearrange("(ch p) d -> p ch d", p=P)
            for h4 in range(4):
                nc.sync.dma_start(out=wvo_v[:, 2 * h4:2 * h4 + 2, :],
                                  in_=wvod_v[:, 2 * h4:2 * h4 + 2, :])
            for k in range(8, 16):
                nc.sync.dma_start(out=sup_v[:, 2 * k:2 * k + 2, :],
                                  in_=supd_v[:, 2 * k:2 * k + 2, :])

            # ---------------- PE warmup ------------------------------------
            # ramp the PE p-state while the wqk DMA streams in
            with nc.named_scope("warmup"):
                for _ in range(8):
                    w_ps = psQ.tile([P, D], f32, tag="ring")
                    nc.tensor.transpose(w_ps[:, 0:P], idf, idf)

            # ---------------- Phase A: Qk = entT.T @ Wqk + bqk -------------
            with nc.named_scope("phaseA"):
                q_ps = psQ.tile([P, D], f32, tag="ring")
                for ch in range(DCH):
                    for nh in range(2):
                        nc.tensor.matmul(
                            q_ps[0:C_LOC, nh * 512:(nh + 1) * 512],
                            entt_sb[:, ch * C_LOC:(ch + 1) * C_LOC],
                            wqk_sb[:, ch * D + nh * 512:ch * D + (nh + 1) * 512],
                            start=(ch == 0), stop=False,
                        )
                for nh in range(2):
                    nc.tensor.matmul(
                        q_ps[0:C_LOC, nh * 512:(nh + 1) * 512],
                        ones_b, bqk_sb[0:1, nh * 512:(nh + 1) * 512],
                        start=False, stop=True,
                    )
                nc.scalar.copy(out=qk_sb, in_=q_ps[0:C_LOC, :])

            res_v = res_d.rearrange("(t p) d -> p t d", p=P)

            # shared scratch (bufs=1: same-engine sequential reuse is fine)
            prod = sbB.tile([P, D], bf16, tag="prod", bufs=1)
            prodg = sbB.tile([P, D], bf16, tag="prodg", bufs=1)

            def b_tile(t, s8, j):
                """scores for tile t -> s8[:, j] (exp/normalize later)."""
                qkb = psQ.tile([P, D], f32, tag="ring")
                for nh in range(2):
                    nc.tensor.matmul(
                        qkb[:, nh * 512:(nh + 1) * 512],
                        ind_sb[:, t * P:(t + 1) * P],
                        qk_sb[:, nh * 512:(nh + 1) * 512],
                        start=True, stop=True,
                    )
                # DVE reads the PSUM qkb directly (no staging drain; the
                # 3-operand stt is 1x on DVE regardless, and walrus rejects
                # it on GPSIMD)
                nc.vector.scalar_tensor_tensor(
                    out=prod, in0=qkb, scalar=INV_SQRT_D,
                    in1=sup_all[:, t * D:(t + 1) * D],
                    op0=MUL, op1=MUL, accum_out=s8[:, j:j + 1])

            def c_group(h, jg, s8):
                """exp + scatter the weight pairs into w_all / w_r."""
                e8 = sbB.tile([P, GSZ], bf16, tag="e8", bufs=2)
                nc.scalar.activation(out=e8, in_=s8, func=EXP)
                b_all = WHALF * h + WPAD * GSZ * jg
                b_r = CPH * h + 2 * GSZ * jg
                nc.vector.tensor_copy(
                    out=w_all[0:K_SHOTS, b_all:b_all + WPAD * GSZ:WPAD],
                    in_=e8[0:K_SHOTS, :])
                nc.vector.tensor_copy(
                    out=w_all[K_SHOTS:P,
                              b_all + 1:b_all + WPAD * (GSZ - 1) + 2:WPAD],
                    in_=e8[K_SHOTS:P, :])
                nc.vector.tensor_copy(
                    out=w_r[0:K_SHOTS, b_r:b_r + 2 * GSZ:2],
                    in_=e8[0:K_SHOTS, :])
                nc.vector.tensor_copy(
                    out=w_r[K_SHOTS:P, b_r + 1:b_r + 2 * GSZ:2],
                    in_=e8[K_SHOTS:P, :])

            def d_tile(h, j, pooled_ps):
                """pooled[32h:32h+32] += w_tile.T @ sup_tile."""
                for nh in range(2):
                    nc.tensor.matmul(
                        pooled_ps[CPH * h:CPH * (h + 1),
                                  nh * 512:(nh + 1) * 512],
                        w_all[:, WHALF * h + 32 * j:WHALF * h + 32 * j + 32],
                        sup_all[:, (16 * h + j) * D + nh * 512:
                                (16 * h + j) * D + (nh + 1) * 512],
                        start=(j == 0), stop=(j == HTILES - 1),
                    )

            def e_half(h, pooled_ps, r_ps):
                """OUT[32h:32h+32] = (pooled/r) @ Wvo + bvo."""
                # r for the whole half in one matmul over the packed bank
                nc.tensor.matmul(
                    r_ps[CPH * h:CPH * (h + 1), :],
                    w_r[:, CPH * h:CPH * (h + 1)], ones_col,
                    start=True, stop=True,
                )
                hs = slice(CPH * h, CPH * (h + 1))
                nc.vector.reciprocal(ri_sb[hs, :], r_ps[hs, :])
                nc.scalar.activation(out=pooled_sb[hs, :], in_=pooled_ps[hs, :],
                                     func=CPY, scale=ri_sb[hs, 0:1])
                for ch in range(DCH):
                    tp = psT.tile([P, CPH], bf16, tag="tp")
                    nc.tensor.transpose(
                        tp, pooled_sb[hs, ch * P:(ch + 1) * P],
                        idb[hs, CPH * h:CPH * (h + 1)],
                    )
                    nc.scalar.copy(
                        out=pooledt_sb[:, ch * C_LOC + CPH * h:
                                       ch * C_LOC + CPH * h + CPH],
                        in_=tp,
                    )
                o_ps = psQ.tile([P, D], f32, tag="ring")
                for ch in range(DCH):
                    for nh in range(2):
                        nc.tensor.matmul(
                            o_ps[hs, nh * 512:(nh + 1) * 512],
                            pooledt_sb[:, ch * C_LOC + CPH * h:
                                       ch * C_LOC + CPH * h + CPH],
                            wvo_sb[:, ch * D + nh * 512:ch * D + (nh + 1) * 512],
                            start=(ch == 0), stop=False,
                        )
                for nh in range(2):
                    nc.tensor.matmul(
                        o_ps[hs, nh * 512:(nh + 1) * 512],
                        ones_b[0:1, 0:CPH], bvo_sb[0:1, nh * 512:(nh + 1) * 512],
                        start=False, stop=True,
                    )
                nc.scalar.copy(out=out_sb[hs, :], in_=o_ps[hs, :])

            def f_tile(h, j):
                """res_tile = sup_tile + OUT[class(row)] (in place)."""
                t = 16 * h + j
                hs = slice(CPH * h, CPH * (h + 1))
                ob = psQ.tile([P, D], f32, tag="ring")
                for nh in range(2):
                    nc.tensor.matmul(
                        ob[:, nh * 512:(nh + 1) * 512],
                        ind_sb[hs, t * P:(t + 1) * P],
                        out_sb[hs, nh * 512:(nh + 1) * 512],
                        start=True, stop=True,
                    )
                st = sup_all[:, t * D:(t + 1) * D]
                if j % 4 == 0:
                    # DVE adds straight from PSUM (fused drain+add)
                    nc.vector.scalar_tensor_tensor(
                        out=st, in0=ob, scalar=1.0, in1=st,
                        op0=MUL, op1=ADD)
                else:
                    ob_sb = sbB.tile([P, D], bf16, tag="ob_sb", bufs=2)
                    nc.scalar.copy(out=ob_sb, in_=ob)
                    if j % 2 == 1:
                        # 2x-mode bf16 add on DVE
                        nc.vector.tensor_tensor(out=st, in0=st, in1=ob_sb,
                                                op=ADD)
                    else:
                        # off-critical-engine add on the idle GPSIMD
                        nc.gpsimd.tensor_tensor(out=st, in0=st, in1=ob_sb,
                                                op=ADD)
                if j % 2 == 1:
                    t0 = t - 1
                    nc.sync.dma_start(
                        out=res_v[:, t0:t0 + 2, :],
                        in_=sup_v[:, t0:t0 + 2, :],
                    )

            pooled_ps = psP.tile([C_LOC, D], f32)
            r_ps = psS.tile([C_LOC, 1], f32)

            # ---------------- half 0: B/C/D, then E ------------------------
            with nc.named_scope("bcdH0"):
                for jg in range(2):
                    s8 = sbB.tile([P, GSZ], f32, tag="s8", bufs=2)
                    for j in range(GSZ):
                        b_tile(jg * GSZ + j, s8, j)
                    c_group(0, jg, s8)
                    for j in range(GSZ):
                        d_tile(0, jg * GSZ + j, pooled_ps)
            with nc.named_scope("e0"):
                e_half(0, pooled_ps, r_ps)

            # ------- F(h0) interleaved with B/C/D(h1), then E(h1) ----------
            with nc.named_scope("f0bcdH1"):
                s8 = sbB.tile([P, GSZ], f32, tag="s8", bufs=2)
                for j in range(GSZ):
                    f_tile(0, j)
                    b_tile(16 + j, s8, j)
                c_group(1, 0, s8)
                s8b = sbB.tile([P, GSZ], f32, tag="s8", bufs=2)
                for j in range(GSZ):
                    f_tile(0, GSZ + j)
                    d_tile(1, j, pooled_ps)
                    b_tile(24 + j, s8b, j)
                c_group(1, 1, s8b)
                for j in range(GSZ):
                    d_tile(1, GSZ + j, pooled_ps)
            with nc.named_scope("e1"):
                e_half(1, pooled_ps, r_ps)

            # ---------------- F(h1) + output DMA ---------------------------
            with nc.named_scope("f1"):
                for j in range(HTILES):
                    f_tile(1, j)

    nc.compile()
    return nc


def _get_nc():
    global _NC_CACHE
    if _NC_CACHE is None:
        _NC_CACHE = _build_nc()
    return _NC_CACHE


def _prep_in_maps(support_features, entity_vectors, support_labels,
                  Wq, bq, Wk, bk, Wv, bv, Wo, bo):
    from ml_dtypes import bfloat16

    sup = np.asarray(support_features, dtype=np.float32)
    ent = np.asarray(entity_vectors, dtype=np.float32)
    labels = np.asarray(support_labels, dtype=np.int32)
    wq = np.asarray(Wq, dtype=np.float32)
    wk = np.asarray(Wk, dtype=np.float32)
    wv = np.asarray(Wv, dtype=np.float32)
    wo = np.asarray(Wo, dtype=np.float32)
    bq_ = np.asarray(bq, dtype=np.float32).reshape(1, D)
    bv_ = np.asarray(bv, dtype=np.float32).reshape(1, D)
    bo_ = np.asarray(bo, dtype=np.float32).reshape(1, D)
    # bk is dropped: it adds a per-class constant to each softmax row.

    # weights-only folding (reparameterization; activation math is on-device)
    wqk = np.ascontiguousarray(wq.T @ wk).astype(bfloat16)
    wvo = np.ascontiguousarray(wv.T @ wo.T).astype(bfloat16)
    bqk = (bq_ @ wk).astype(bfloat16)
    bvo = (bv_ @ wo.T + bo_).astype(bfloat16)

    expected = np.arange(NK, dtype=np.int32) // K_SHOTS
    assert np.array_equal(labels, expected), (
        "kernel assumes exactly K_SHOTS contiguous samples per class "
        "(labels == arange(NK)//K_SHOTS)"
    )

    sup_bf = sup.astype(bfloat16)
    in_maps = []
    for c in range(N_CORES):
        lab_loc = labels[c * R_LOC:(c + 1) * R_LOC] - c * C_LOC
        ind = (lab_loc[None, :] ==
               np.arange(C_LOC, dtype=np.int32)[:, None]).astype(bfloat16)
        in_maps.append({
            "sup": np.ascontiguousarray(sup_bf[c * R_LOC:(c + 1) * R_LOC]),
            "entt": np.ascontiguousarray(
                ent[c * C_LOC:(c + 1) * C_LOC].T).astype(bfloat16),
            "ind": np.ascontiguousarray(ind),
            "wqk": wqk, "wvo": wvo, "bqk": bqk, "bvo": bvo,
        })
    return in_maps


def _run(in_maps, **kwargs):
    from concourse.bass_utils import run_bass_kernel_spmd
    nc = _get_nc()
    return run_bass_kernel_spmd(nc, in_maps, core_ids=list(range(N_CORES)),
                                **kwargs)


def kernel(support_features, entity_vectors, support_labels,
           Wq, bq, Wk, bk, Wv, bv, Wo, bo):
    in_maps = _prep_in_maps(support_features, entity_vectors, support_labels,
                            Wq, bq, Wk, bk, Wv, bv, Wo, bo)
    r = _run(in_maps)
    return np.concatenate(
        [np.asarray(r.results[c]["res"], dtype=np.float32)
         for c in range(N_CORES)], axis=0)
